# revision 29
# baseline (speedup 1.0000x reference)
"""GAT layer (BatchNorm -> GATConv -> head-mean -> ELU -> per-graph Conv1d)
on 8 Trainium2 NeuronCores via Bass/Tile.

v2 design: the edge gather is done host-side (pure index manipulation, like
the baseline's xperm/srcs prep): x[src] rows are laid out in the padded,
destination-rank-sorted edge stream order.  The device then:
  1. computes BN stats and folds BN + attention vectors into small
     broadcast constants (va, vd, ca+cd),
  2. streams the pre-gathered raw x edge rows with direct DMA and computes
     attention weights + accumulates per-destination segment sums as dense
     "round" adds (no indirect DMA in the main loop),
  3. corrects the padding slots' denominator contribution analytically
     (pad slots have x=0 so their exp(lrelu(adst2)) weight is computable
     densely per rank),
  4. normalizes, projects through a BN-folded weight matrix (block-diag x4
     for partition efficiency), adds bias, applies ELU, and
  5. writes normalized rows densely in rank order, gathers them into
     per-graph padded position order (190 indirect DMAs, issued upfront
     and pipelined with the projection), and runs the Conv1d as a
     residue-5 decomposition: t = 5q + r gives M=40 output rows x
     N=3200 columns in 14 K-passes of 120 (fp16 operands, fp32 psum).

All FP compute on tensor data happens on-device; host work is index
manipulation and pure weight-layout transforms.
"""

import sys

sys.path.insert(0, "/opt/trn_rl_repo")

import numpy as np
from contextlib import ExitStack

import concourse.bass as bass
import concourse.bacc as bacc
import concourse.tile as tile
from concourse import mybir
from concourse.masks import make_identity
from concourse.bass_utils import run_bass_kernel_spmd

F32 = mybir.dt.float32
BF16 = mybir.dt.float16
I32 = mybir.dt.int32
AF = mybir.ActivationFunctionType
OP = mybir.AluOpType

N = 190464
FIN = 5
H = 4
C = 24
NPG = 186          # nodes per graph
B = 1024           # graphs
NCORES = 8
GPC = B // NCORES  # 128 graphs per core
NL = N // NCORES   # 23808 local nodes per core
NBLK = NL // 128   # 186 rank blocks of 128
MB = N // 128      # 1488 nodes per partition in the flat x layout
KCONV = 62
COUT = 8
TOUT = NPG - KCONV + 1   # 125
PADG = 190               # per-graph padded length (5*38, conv shift head-room)
NPOS = GPC * PADG        # 24320 padded node positions per core
NPT = NPOS // 128        # 190 position tiles
G_CH = 128               # max edge tiles per main-loop chunk
QD = TOUT // 5           # 25 output positions per residue
NP5 = 14                 # conv K passes (j = 5p + jj, j in 0..69)
EPS = 1e-5


# --------------------------------------------------------------------------
# host-side sharding / ordering (pure index manipulation)
# --------------------------------------------------------------------------
def _host_prep(edge_index, x):
    src_g = np.asarray(edge_index[0], dtype=np.int64)
    dst_g = np.asarray(edge_index[1], dtype=np.int64)
    xz = np.concatenate([np.asarray(x, np.float32),
                         np.zeros((1, FIN), np.float32)], axis=0)
    cores = []
    maxdeg = 0
    for k in range(NCORES):
        lo = k * NL
        m = (dst_g >= lo) & (dst_g < lo + NL)
        es = np.concatenate([src_g[m], np.arange(lo, lo + NL, dtype=np.int64)])
        ed = np.concatenate([dst_g[m] - lo, np.arange(NL, dtype=np.int64)])
        deg = np.bincount(ed, minlength=NL)
        node_of_rank = np.argsort(-deg, kind="stable")
        rank_of_node = np.empty(NL, dtype=np.int64)
        rank_of_node[node_of_rank] = np.arange(NL)
        r_e = rank_of_node[ed]
        perm = np.argsort(r_e, kind="stable")
        es_s = es[perm]
        r_s = r_e[perm]
        cnt = deg[node_of_rank]          # per-rank degree, descending
        starts = np.zeros(NL, dtype=np.int64)
        starts[1:] = np.cumsum(cnt)[:-1]
        j_s = np.arange(es_s.size, dtype=np.int64) - starts[r_s]
        maxdeg = max(maxdeg, int(cnt[0]))
        cores.append(dict(es_s=es_s, r_s=r_s, j_s=j_s, cnt=cnt,
                          node_of_rank=node_of_rank))

    # global (SPMD-identical) round sizes: K_j = #nodes with deg > j
    kmax = np.zeros(maxdeg, dtype=np.int64)
    for c in cores:
        kj = np.searchsorted(-c["cnt"], -np.arange(maxdeg), side="left")
        kmax = np.maximum(kmax, kj)
    r_tiles = (kmax + 127) // 128            # tiles per round
    r_edges = r_tiles * 128
    round_base = np.zeros(maxdeg + 1, dtype=np.int64)
    round_base[1:] = np.cumsum(r_edges)
    e_pad = int(round_base[-1])
    nt_total = e_pad // 128

    # chunk schedule (identical across cores): (t0, c0, nt)
    chunks = []
    t0 = 0
    for j in range(maxdeg):
        rem = int(r_tiles[j])
        c0 = 0
        while rem:
            nt = min(G_CH, rem)
            chunks.append((t0, c0, nt))
            t0 += nt
            c0 += nt
            rem -= nt
    assert t0 == nt_total

    per_core = []
    for c in cores:
        stream = np.full(e_pad, N, dtype=np.int64)    # zero row for padding
        pos = round_base[c["j_s"]] + c["r_s"]
        stream[pos] = c["es_s"]
        xe = xz[stream]                               # [e_pad, 5]
        xe = np.ascontiguousarray(
            xe.reshape(nt_total, 128, FIN).transpose(1, 0, 2).reshape(
                128, nt_total * FIN))

        # per-rank padding-slot count: rank r is padded in rounds
        # j in [cnt[r], maxdeg) where r < r_edges[j]
        npad = np.zeros(NL, dtype=np.int64)
        cnts = c["cnt"]
        for j in range(maxdeg):
            lo_r = np.searchsorted(-cnts, -j, side="left")  # first rank w/ cnt<=j
            hi_r = int(r_edges[j])
            if hi_r > lo_r:
                npad[lo_r:hi_r] += 1
        npad2 = np.ascontiguousarray(
            npad.reshape(NBLK, 128).T).astype(np.float32)

        nor = c["node_of_rank"]
        rank_of_node = np.empty(NL, dtype=np.int64)
        rank_of_node[nor] = np.arange(NL)
        gid = np.full(NPOS, NL, dtype=np.int64)       # pad -> zero row NL
        posa = np.arange(NPOS)
        g = posa // PADG
        s = posa % PADG
        real = s < NPG
        gid[real] = rank_of_node[g[real] * NPG + s[real]]
        gidx = np.ascontiguousarray(
            gid.reshape(NPT, 128).T).astype(np.int32)

        per_core.append(dict(xe=xe, npad=npad2, gidx=gidx,
                             node_of_rank=nor))

    amask = np.zeros((H * C, H), dtype=np.float32)
    for h in range(H):
        amask[h * C:(h + 1) * C, h] = 1.0
    return per_core, chunks, nt_total, amask


def _w3_layout20(lin_w):
    """Rows (h,f) = lin_w[h*24+c', f] (pure layout)."""
    w3 = np.zeros((4 * FIN, C), dtype=np.float32)
    for h in range(H):
        w3[h * FIN:(h + 1) * FIN, :] = lin_w[h * C:(h + 1) * C, :].T
    return w3


def _conv_w5b(cw):
    """conv_w[o, ci, k] -> w5b[(jj,ci), (p,(r,o))]: nonzero iff k=5p+jj-r
    in [0, KCONV). Pure layout."""
    w = np.zeros((5 * C, NP5 * 40), dtype=np.float32)
    for p in range(NP5):
        for jj in range(5):
            for r in range(5):
                k = 5 * p + jj - r
                if 0 <= k < KCONV:
                    w[jj * C:(jj + 1) * C, p * 40 + r * COUT:(p * 40 + (r + 1) * COUT)] = \
                        cw[:, :, k].T
    return w


# --------------------------------------------------------------------------
# device program
# --------------------------------------------------------------------------
def _build(nt_total, chunks):
    nc = bacc.Bacc(None, target_bir_lowering=False)
    x_d = nc.declare_dram_parameter("x", [N, FIN], F32, isOutput=False)
    xperm_d = nc.declare_dram_parameter("xperm", [128, NBLK * FIN], F32, False)
    xe_d = nc.declare_dram_parameter("xe", [128, nt_total * FIN], F32, False)
    npad_d = nc.declare_dram_parameter("npad", [128, NBLK], F32, False)
    gidx_d = nc.declare_dram_parameter("gidx", [128, NPT], I32, False)
    amask_d = nc.declare_dram_parameter("amask", [H * C, H], F32, False)
    gam_d = nc.declare_dram_parameter("bn_gamma", [FIN], F32, False)
    bet_d = nc.declare_dram_parameter("bn_beta", [FIN], F32, False)
    lw_d = nc.declare_dram_parameter("lin_w", [H * C, FIN], F32, False)
    lwT_d = nc.declare_dram_parameter("lin_wT", [FIN, H * C], F32, False)
    csel_d = nc.declare_dram_parameter("csel", [H * C, C], F32, False)
    asc_d = nc.declare_dram_parameter("att_src", [H, C], F32, False)
    adc_d = nc.declare_dram_parameter("att_dst", [H, C], F32, False)
    gb_d = nc.declare_dram_parameter("gat_bias", [C], F32, False)
    w3c4_d = nc.declare_dram_parameter("w3cat4", [80, 96], F32, False)
    w5b_d = nc.declare_dram_parameter("w5b", [5 * C, NP5 * 40], F32, False)
    cb40_d = nc.declare_dram_parameter("conv_b40", [40], F32, False)
    # out in [(r,o), (g,q)] layout; host de-interleaves t = 5q + r
    out_d = nc.declare_dram_parameter("out", [40, GPC * QD], F32,
                                      isOutput=True)

    spillA = nc.dram_tensor("spillA", [NL + 128, 4 * FIN], BF16)
    wscr = nc.dram_tensor("wscr", [FIN, 2 * H], F32)

    with tile.TileContext(nc) as tc, ExitStack() as ctx:
        cpool = ctx.enter_context(tc.tile_pool(name="const", bufs=1))

        # ---------- persistent constants ----------
        ident = cpool.tile([128, 128], F32)
        make_identity(nc, ident[:])
        ones_col = cpool.tile([128, 1], F32)
        nc.vector.memset(ones_col[:], 1.0)
        ones_row = cpool.tile([1, 128], F32)
        nc.vector.memset(ones_row[:], 1.0)

        acc20 = cpool.tile([128, NBLK * 4 * FIN], F32)
        nc.vector.memset(acc20[:], 0.0)
        accW = cpool.tile([128, NBLK * H], F32)
        nc.vector.memset(accW[:], 0.0)
        adst2 = cpool.tile([128, NBLK * H], F32)
        adst2h = cpool.tile([128, NBLK * H], BF16)
        vah = cpool.tile([128, 4 * FIN], BF16)

        gidx_sb = cpool.tile([128, NPT], I32)
        nc.sync.dma_start(out=gidx_sb[:], in_=gidx_d[:, :])
        npad_sb = cpool.tile([128, NBLK], F32)
        nc.sync.dma_start(out=npad_sb[:], in_=npad_d[:, :])

        identb = cpool.tile([128, 128], BF16)
        nc.vector.tensor_copy(out=identb[:], in_=ident[:])
        w3p4 = cpool.tile([80, 96], F32)       # BN-folded block-diag proj
        w3p4b = cpool.tile([80, 96], BF16)
        wc5f = cpool.tile([5 * C, NP5 * 40], F32)
        nc.sync.dma_start(out=wc5f[:], in_=w5b_d[:, :])
        wc5 = cpool.tile([5 * C, NP5 * 40], BF16)
        nc.vector.tensor_copy(out=wc5[:], in_=wc5f[:])
        cb40 = cpool.tile([40, 1], F32)
        nc.sync.dma_start(out=cb40[:], in_=cb40_d[:, None])
        cb4 = cpool.tile([96, 1], F32)         # gat out bias x4 (h-mean folded)

        b54 = cpool.tile([128, 54], F32)  # [s(5) t(5) va(20) vd(20) cacd(4)]

        # ---------- setup phase: BN stats, folded vectors, adst2 ----------
        with tc.tile_pool(name="setup", bufs=1) as spool, \
             tc.tile_pool(name="spsum", bufs=1, space="PSUM") as sps:
            # zero the dummy rows NL..NL+127 of spillA (pad positions)
            zpad = spool.tile([128, 4 * FIN], BF16)
            nc.vector.memset(zpad[:], 0.0)
            nc.sync.dma_start(out=spillA[NL:NL + 128, :], in_=zpad[:])

            xsb = spool.tile([128, MB * FIN], F32)
            nc.sync.dma_start(
                out=xsb[:],
                in_=x_d[:, :].rearrange("(p m) f -> p (m f)", p=128))
            tmp = spool.tile([128, MB * FIN], F32)

            # per-partition partial sums of x and x^2  -> [128, 10]
            xpart = spool.tile([128, 10], F32)
            nc.scalar.activation(out=tmp[:], in_=xsb[:], func=AF.Square)
            nc.vector.tensor_reduce(
                out=xpart[:, 0:FIN],
                in_=xsb[:].rearrange("p (m f) -> p f m", f=FIN),
                axis=mybir.AxisListType.X, op=OP.add)
            nc.vector.tensor_reduce(
                out=xpart[:, FIN:2 * FIN],
                in_=tmp[:].rearrange("p (m f) -> p f m", f=FIN),
                axis=mybir.AxisListType.X, op=OP.add)
            sums_ps = sps.tile([1, 10], F32)
            nc.tensor.matmul(out=sums_ps[:], lhsT=ones_col[:], rhs=xpart[:],
                             start=True, stop=True)

            st1 = spool.tile([1, 32], F32)
            w2 = spool.tile([1, 54], F32)
            nc.vector.tensor_copy(out=st1[:, 0:10], in_=sums_ps[:])
            nc.vector.tensor_scalar_mul(out=st1[:, 0:5], in0=st1[:, 0:5],
                                        scalar1=1.0 / N)          # mu
            nc.vector.tensor_scalar_mul(out=st1[:, 5:10], in0=st1[:, 5:10],
                                        scalar1=1.0 / N)          # E[x^2]
            nc.vector.tensor_tensor(out=st1[:, 10:15], in0=st1[:, 0:5],
                                    in1=st1[:, 0:5], op=OP.mult)  # mu^2
            nc.vector.tensor_tensor(out=st1[:, 10:15], in0=st1[:, 5:10],
                                    in1=st1[:, 10:15], op=OP.subtract)  # var
            nc.vector.tensor_scalar_add(out=st1[:, 15:20],
                                        in0=st1[:, 10:15], scalar1=EPS)
            nc.scalar.activation(out=st1[:, 15:20], in_=st1[:, 15:20],
                                 func=AF.Sqrt)
            nc.vector.reciprocal(out=st1[:, 10:15], in_=st1[:, 15:20])  # rstd
            gsb = spool.tile([1, FIN], F32)
            bsb = spool.tile([1, FIN], F32)
            nc.sync.dma_start(out=gsb[:], in_=gam_d[None, :])
            nc.sync.dma_start(out=bsb[:], in_=bet_d[None, :])
            nc.vector.tensor_tensor(out=w2[:, 0:5], in0=gsb[:],
                                    in1=st1[:, 10:15], op=OP.mult)  # s
            nc.vector.tensor_tensor(out=st1[:, 20:25], in0=st1[:, 0:5],
                                    in1=w2[:, 0:5], op=OP.mult)     # mu*s
            nc.vector.tensor_tensor(out=w2[:, 5:10], in0=bsb[:],
                                    in1=st1[:, 20:25], op=OP.subtract)  # t

            # wa / wd via block-masked attention matmul (as baseline)
            attfs = spool.tile([H * C, 1], F32)
            attfd = spool.tile([H * C, 1], F32)
            nc.sync.dma_start(out=attfs[:],
                              in_=asc_d[:, :].rearrange("h c -> (h c)")[:, None])
            nc.sync.dma_start(out=attfd[:],
                              in_=adc_d[:, :].rearrange("h c -> (h c)")[:, None])
            amk = spool.tile([H * C, H], F32)
            nc.sync.dma_start(out=amk[:], in_=amask_d[:, :])
            a2 = spool.tile([H * C, 2 * H], F32)
            nc.vector.tensor_tensor(out=a2[:, 0:H],
                                    in0=attfs[:].to_broadcast([H * C, H]),
                                    in1=amk[:], op=OP.mult)
            nc.vector.tensor_tensor(out=a2[:, H:2 * H],
                                    in0=attfd[:].to_broadcast([H * C, H]),
                                    in1=amk[:], op=OP.mult)
            lwsb = spool.tile([H * C, FIN], F32)
            nc.sync.dma_start(out=lwsb[:], in_=lw_d[:, :])
            wps = sps.tile([FIN, 2 * H], F32)
            nc.tensor.matmul(out=wps[:], lhsT=lwsb[:], rhs=a2[:],
                             start=True, stop=True)
            wsb = spool.tile([FIN, 2 * H], F32)
            nc.vector.tensor_copy(out=wsb[:], in_=wps[:])
            nc.sync.dma_start(out=wscr[:, :], in_=wsb[:])
            tc.strict_bb_all_engine_barrier()   # wscr DRAM RAW
            # reload wa/wd in (h major, f minor) order: addr = f*8 + hh
            wflat = wscr[:, :].rearrange("f hh -> (f hh)")
            wad = spool.tile([1, 40], F32)
            nc.sync.dma_start(
                out=wad[:, 0:20],
                in_=wflat[None, :].rearrange("p (f hh) -> p hh f", hh=8)[:, 0:4, :])
            nc.sync.dma_start(
                out=wad[:, 20:40],
                in_=wflat[None, :].rearrange("p (f hh) -> p hh f", hh=8)[:, 4:8, :])
            # va = wa*s, vd = wd*s (per-head, f minor)
            s_bc4 = w2[:, 0:5][:, None, :].to_broadcast([1, 4, 5])
            nc.vector.tensor_tensor(
                out=w2[:, 10:30].rearrange("p (h f) -> p h f", f=5),
                in0=wad[:, 0:20].rearrange("p (h f) -> p h f", f=5),
                in1=s_bc4, op=OP.mult)
            nc.vector.tensor_tensor(
                out=w2[:, 30:50].rearrange("p (h f) -> p h f", f=5),
                in0=wad[:, 20:40].rearrange("p (h f) -> p h f", f=5),
                in1=s_bc4, op=OP.mult)
            # ca+cd: (wa+wd)*t summed over f
            t_bc4 = w2[:, 5:10][:, None, :].to_broadcast([1, 4, 5])
            wsum = spool.tile([1, 20], F32)
            nc.vector.tensor_tensor(out=wsum[:], in0=wad[:, 0:20],
                                    in1=wad[:, 20:40], op=OP.add)
            nc.vector.tensor_tensor(
                out=wsum[:].rearrange("p (h f) -> p h f", f=5),
                in0=wsum[:].rearrange("p (h f) -> p h f", f=5),
                in1=t_bc4, op=OP.mult)
            nc.vector.tensor_reduce(
                out=w2[:, 50:54],
                in_=wsum[:].rearrange("p (h f) -> p h f", f=5),
                axis=mybir.AxisListType.X, op=OP.add)

            # broadcast [1,54] -> [128,54] with a K=1 matmul
            b54_ps = sps.tile([128, 54], F32)
            nc.tensor.matmul(out=b54_ps[:], lhsT=ones_row[:], rhs=w2[:],
                             start=True, stop=True)
            nc.vector.tensor_copy(out=b54[:], in_=b54_ps[:])

            # s80 partition vector (s pattern x16) via K=1 matmul
            s80row = spool.tile([1, 80], F32)
            nc.vector.tensor_copy(
                out=s80row[:].rearrange("p (i f) -> p i f", f=5),
                in_=w2[:, 0:5][:, None, :].to_broadcast([1, 16, 5]))
            s80ps = sps.tile([80, 1], F32)
            ones1 = spool.tile([1, 1], F32)
            nc.vector.memset(ones1[:], 1.0)
            nc.tensor.matmul(out=s80ps[:], lhsT=s80row[:], rhs=ones1[:],
                             start=True, stop=True)
            s80 = spool.tile([80, 1], F32)
            nc.vector.tensor_copy(out=s80[:], in_=s80ps[:])
            # w3p4 = w3cat4 * s80 (BN scale folded into projection)
            w3c4 = spool.tile([80, 96], F32)
            nc.sync.dma_start(out=w3c4[:], in_=w3c4_d[:, :])
            nc.vector.tensor_tensor(out=w3p4[:], in0=w3c4[:],
                                    in1=s80[:].to_broadcast([80, 96]),
                                    op=OP.mult)
            nc.vector.tensor_copy(out=w3p4b[:], in_=w3p4[:])

            # cbase = 0.25 * sum_h (lin_w[(h,c)] . t) + gat_bias
            tcolps = sps.tile([FIN, 1], F32)
            nc.tensor.matmul(out=tcolps[:], lhsT=w2[:, 5:10], rhs=ones1[:],
                             start=True, stop=True)
            tcol = spool.tile([FIN, 1], F32)
            nc.vector.tensor_copy(out=tcol[:], in_=tcolps[:])
            lwT = spool.tile([FIN, H * C], F32)
            nc.sync.dma_start(out=lwT[:], in_=lwT_d[:, :])
            lwtps = sps.tile([H * C, 1], F32)
            nc.tensor.matmul(out=lwtps[:], lhsT=lwT[:], rhs=tcol[:],
                             start=True, stop=True)
            lwt96 = spool.tile([H * C, 1], F32)
            nc.vector.tensor_copy(out=lwt96[:], in_=lwtps[:])
            cselsb = spool.tile([H * C, C], F32)
            nc.sync.dma_start(out=cselsb[:], in_=csel_d[:, :])
            cbps = sps.tile([C, 1], F32)
            nc.tensor.matmul(out=cbps[:], lhsT=cselsb[:], rhs=lwt96[:],
                             start=True, stop=True)
            gb24 = spool.tile([C, 1], F32)
            nc.sync.dma_start(out=gb24[:], in_=gb_d[:, None])
            cbase = spool.tile([C, 1], F32)
            nc.scalar.activation(out=cbase[:], in_=cbps[:], func=AF.Identity,
                                 scale=0.25, bias=gb24[:])
            for i in range(4):
                nc.sync.dma_start(out=cb4[i * C:(i + 1) * C, :], in_=cbase[:])

            # adst2[p, (blk,h)] = sum_f vd[h,f]*xperm[p,blk,f] + (ca+cd)[h]
            xpb = spool.tile([128, NBLK * FIN], F32)
            nc.sync.dma_start(out=xpb[:], in_=xperm_d[:, :])
            xp_v = xpb[:].rearrange("p (m f) -> p m f", f=FIN)
            for h in range(H):
                vd_bc = b54[:, 30 + FIN * h:30 + FIN * (h + 1)]
                vd_bc = vd_bc[:, None, :].to_broadcast([128, NBLK, FIN])
                nc.vector.tensor_tensor(
                    out=tmp[:, 0:NBLK * FIN].rearrange("p (m f) -> p m f", f=FIN),
                    in0=xp_v, in1=vd_bc, op=OP.mult)
                nc.vector.tensor_reduce(
                    out=adst2[:].rearrange("p (m h) -> p m h", h=H)[:, :, h],
                    in_=tmp[:, 0:NBLK * FIN].rearrange("p (m f) -> p m f", f=FIN),
                    axis=mybir.AxisListType.X, op=OP.add)
            cacd_bc = b54[:, 50:54][:, None, :].to_broadcast([128, NBLK, H])
            nc.vector.tensor_tensor(
                out=adst2[:].rearrange("p (m h) -> p m h", h=H),
                in0=adst2[:].rearrange("p (m h) -> p m h", h=H),
                in1=cacd_bc, op=OP.add)
            nc.vector.tensor_copy(out=adst2h[:], in_=adst2[:])
            nc.vector.tensor_copy(out=vah[:], in_=b54[:, 10:30])

        # ---------- main edge loop (no indirect DMA) ----------
        with tc.tile_pool(name="gat", bufs=3) as gpool, \
             tc.tile_pool(name="mwork", bufs=2) as mpool, \
             nc.allow_low_precision(reason="fp16 attention path, 2e-2 gate"):
            for (t0, c0, nt) in chunks:
                xef = gpool.tile([128, G_CH * FIN], F32, tag="xef")
                nc.sync.dma_start(out=xef[:, 0:nt * FIN],
                                  in_=xe_d[:, t0 * FIN:(t0 + nt) * FIN])
                xet = gpool.tile([128, G_CH * FIN], BF16, tag="xet")
                nc.gpsimd.tensor_copy(out=xet[:, 0:nt * FIN],
                                      in_=xef[:, 0:nt * FIN])
                xev = xet[:].rearrange("p (g f) -> p g f", f=FIN)
                tm = mpool.tile([128, G_CH * 4 * FIN], BF16, tag="tm")
                tmv = tm[:].rearrange("p (g h f) -> p g h f", h=H, f=FIN)
                va_bc = vah[:].rearrange(
                    "p (h f) -> p h f", f=FIN)[:, None, :, :]
                nc.vector.tensor_tensor(
                    out=tmv[:, 0:nt],
                    in0=xev[:, 0:nt, None, :].to_broadcast([128, nt, H, FIN]),
                    in1=va_bc.to_broadcast([128, nt, H, FIN]),
                    op=OP.mult)
                zt = mpool.tile([128, G_CH * H], BF16, tag="zt")
                nc.vector.tensor_reduce(
                    out=zt[:, 0:nt * H],
                    in_=tm[:, 0:nt * 4 * FIN].rearrange(
                        "p (gh f) -> p gh f", f=FIN),
                    axis=mybir.AxisListType.X, op=OP.add)
                nc.vector.tensor_tensor(
                    out=zt[:, 0:nt * H], in0=zt[:, 0:nt * H],
                    in1=adst2h[:, c0 * H:(c0 + nt) * H], op=OP.add)
                wt = mpool.tile([128, G_CH * H], BF16, tag="wt")
                # leaky_relu(z, 0.2) = max(0.2*z, z)
                nc.vector.scalar_tensor_tensor(
                    out=wt[:, 0:nt * H], in0=zt[:, 0:nt * H], scalar=0.2,
                    in1=zt[:, 0:nt * H], op0=OP.mult, op1=OP.max)
                nc.scalar.activation(out=wt[:, 0:nt * H], in_=wt[:, 0:nt * H],
                                     func=AF.Exp)
                ut = mpool.tile([128, G_CH * 4 * FIN], BF16, tag="ut")
                uv = ut[:].rearrange("p (g h f) -> p g h f", h=H, f=FIN)
                wv = wt[:].rearrange("p (g h) -> p g h", h=H)
                nc.vector.tensor_tensor(
                    out=uv[:, 0:nt],
                    in0=wv[:, 0:nt, :, None].to_broadcast([128, nt, H, FIN]),
                    in1=xev[:, 0:nt, None, :].to_broadcast([128, nt, H, FIN]),
                    op=OP.mult)
                nc.vector.tensor_tensor(
                    out=acc20[:, c0 * 20:(c0 + nt) * 20],
                    in0=acc20[:, c0 * 20:(c0 + nt) * 20],
                    in1=ut[:, 0:nt * 20],
                    op=OP.add)
                nc.gpsimd.tensor_tensor(
                    out=accW[:, c0 * H:(c0 + nt) * H],
                    in0=accW[:, c0 * H:(c0 + nt) * H],
                    in1=wt[:, 0:nt * H],
                    op=OP.add)

        # ---------- normalize + scatter to position order ----------
        with tc.tile_pool(name="m2", bufs=2) as m2pool:
            # subtract padding-slot weights: wpad = exp(lrelu(adst2))
            wp = m2pool.tile([128, NBLK * H], F32, tag="wp")
            nc.vector.scalar_tensor_tensor(
                out=wp[:], in0=adst2[:], scalar=0.2,
                in1=adst2[:], op0=OP.mult, op1=OP.max)
            nc.scalar.activation(out=wp[:], in_=wp[:], func=AF.Exp)
            wpv = wp[:].rearrange("p (m h) -> p m h", h=H)
            nc.vector.tensor_tensor(
                out=wpv,
                in0=wpv,
                in1=npad_sb[:][:, :, None].to_broadcast([128, NBLK, H]),
                op=OP.mult)
            nc.vector.tensor_tensor(out=accW[:], in0=accW[:], in1=wp[:],
                                    op=OP.subtract)
            # U' = acc20 * (0.25 / denom)
            rden = m2pool.tile([128, NBLK * H], F32, tag="rden")
            nc.vector.reciprocal(out=rden[:], in_=accW[:])
            rd_v = rden[:].rearrange("p (c h) -> p c h", h=H)
            acc_v = acc20[:].rearrange("p (c h f) -> p c h f", h=H, f=FIN)
            accb16 = m2pool.tile([128, NBLK * 4 * FIN], BF16, tag="accb16")
            nc.vector.scalar_tensor_tensor(
                out=acc_v, in0=acc_v, scalar=0.25,
                in1=rd_v[:, :, :, None].to_broadcast([128, NBLK, H, FIN]),
                op0=OP.mult, op1=OP.mult)
            nc.vector.tensor_copy(out=accb16[:], in_=acc20[:])
            # dense write in rank order; position gathers read it back
            nc.sync.dma_start(
                out=spillA[0:NL, :].rearrange("(m p) c -> p m c", p=128),
                in_=accb16[:].rearrange("p (m c) -> p m c", c=4 * FIN))

        tc.strict_bb_all_engine_barrier()   # spillA DRAM RAW before gathers

        # ---------- reload, project, ELU into conv layout; conv ----------
        with tc.tile_pool(name="tail", bufs=1) as tpool, \
             tc.tile_pool(name="tl2", bufs=3) as tl2:
            v5 = tpool.tile([5 * C, NPOS], BF16)
            rl = tpool.tile([128, NPT * 20], BF16)
            for m in range(NPT):
                nc.gpsimd.indirect_dma_start(
                    out=rl[:, m * 20:(m + 1) * 20],
                    out_offset=None,
                    in_=spillA[:, :],
                    in_offset=bass.IndirectOffsetOnAxis(
                        ap=gidx_sb[:, m:m + 1], axis=0),
                )
            with tc.tile_pool(name="tlps", bufs=3, space="PSUM") as tlps:
                for m0 in range(0, NPT, 4):
                    gm = min(4, NPT - m0)
                    tps = tlps.tile([80, 128], BF16, tag="tps")
                    nc.tensor.transpose(
                        out=tps[0:gm * 20, :],
                        in_=rl[:, m0 * 20:(m0 + gm) * 20],
                        identity=identb[:])
                    trs = tl2.tile([80, 128], BF16, tag="trs")
                    nc.vector.tensor_copy(out=trs[0:gm * 20, :],
                                          in_=tps[0:gm * 20, :])
                    pm = tlps.tile([96, 128], F32, tag="pm")
                    nc.tensor.matmul(out=pm[0:gm * C, :],
                                     lhsT=w3p4b[0:gm * 20, 0:gm * C],
                                     rhs=trs[0:gm * 20, :],
                                     start=True, stop=True)
                    t1 = tl2.tile([96, 128], BF16, tag="t1")
                    nc.scalar.activation(out=t1[0:gm * C, :],
                                         in_=pm[0:gm * C, :],
                                         func=AF.Identity,
                                         bias=cb4[0:gm * C, :])
                    # ELU = max(x,0) + exp(min(x,0)) - 1
                    rp = tl2.tile([96, 128], BF16, tag="rp")
                    nc.vector.tensor_scalar_max(out=rp[0:gm * C, :],
                                                in0=t1[0:gm * C, :],
                                                scalar1=0.0)
                    nc.vector.tensor_scalar_min(out=t1[0:gm * C, :],
                                                in0=t1[0:gm * C, :],
                                                scalar1=0.0)
                    nc.scalar.activation(out=t1[0:gm * C, :],
                                         in_=t1[0:gm * C, :], func=AF.Exp)
                    nc.vector.scalar_tensor_tensor(
                        out=t1[0:gm * C, :], in0=t1[0:gm * C, :], scalar=-1.0,
                        in1=rp[0:gm * C, :], op0=OP.add, op1=OP.add)
                    for i in range(gm):
                        eng = nc.sync if i % 2 == 0 else nc.scalar
                        eng.dma_start(
                            out=v5[0:C, (m0 + i) * 128:(m0 + i + 1) * 128],
                            in_=t1[i * C:(i + 1) * C, :])

            # 4 shifted copies for the (ci, jj) contraction rows, chunked
            # across the three DMA-capable engine queues
            dmaengs = [nc.sync, nc.scalar, nc.gpsimd]
            CHK = 5
            csz = (NPOS + CHK - 1) // CHK
            ei = 0
            for kk in range(1, 5):
                for cc in range(CHK):
                    a = cc * csz
                    b = min(NPOS - kk, (cc + 1) * csz)
                    if b <= a:
                        continue
                    dmaengs[ei % 3].dma_start(
                        out=v5[kk * C:(kk + 1) * C, a:b],
                        in_=v5[0:C, a + kk:b + kk])
                    ei += 1
            tc.strict_bb_all_engine_barrier()

            # conv: out[(r,o), (g,q)] accumulated over NP5 K-passes
            gsz = [20, 20, 20, 20, 20, 20, 8]
            g0s = [0, 20, 40, 60, 80, 100, 120]
            v5v = v5[:].rearrange("k (g t) -> k g t", t=PADG)
            with tc.tile_pool(name="cps", bufs=1, space="PSUM") as cps:
                pcs = []
                for ci in range(7):
                    pc_t = cps.tile([40, 512], F32, tag=f"pc{ci}")
                    pcs.append(pc_t)
                for p in range(NP5):
                    for ci in range(7):
                        g0, gn = g0s[ci], gsz[ci]
                        nc.tensor.matmul(
                            out=pcs[ci][:, 0:gn * QD],
                            lhsT=wc5[:, p * 40:(p + 1) * 40],
                            rhs=v5v[:, g0:g0 + gn, 5 * p:5 * p + 121:5],
                            start=(p == 0), stop=(p == NP5 - 1))
                for ci in range(7):
                    g0, gn = g0s[ci], gsz[ci]
                    osb = tl2.tile([40, 512], F32, tag="osb")
                    nc.scalar.activation(out=osb[:, 0:gn * QD],
                                         in_=pcs[ci][:, 0:gn * QD],
                                         func=AF.Lrelu, bias=cb40[:],
                                         alpha=0.01)
                    nc.sync.dma_start(
                        out=out_d[:, g0 * QD:(g0 + gn) * QD],
                        in_=osb[:, 0:gn * QD])

    nc.compile()
    return nc


# --------------------------------------------------------------------------
# entry point
# --------------------------------------------------------------------------
def kernel(**inputs):
    x = np.ascontiguousarray(np.asarray(inputs["x"], dtype=np.float32))
    edge_index = np.asarray(inputs["edge_index"])
    per_core, chunks, nt_total, amask = _host_prep(edge_index, x)

    nc = _build(nt_total, chunks)

    lin_w = np.ascontiguousarray(np.asarray(inputs["lin_w"], np.float32))
    w3c20 = _w3_layout20(lin_w)
    w3c4 = np.zeros((80, 96), np.float32)
    for i in range(4):
        w3c4[i * 20:(i + 1) * 20, i * 24:(i + 1) * 24] = w3c20
    csel = np.tile(np.eye(C, dtype=np.float32), (H, 1))
    common = dict(
        x=x,
        amask=amask,
        bn_gamma=np.asarray(inputs["bn_gamma"], np.float32),
        bn_beta=np.asarray(inputs["bn_beta"], np.float32),
        lin_w=lin_w,
        lin_wT=np.ascontiguousarray(lin_w.T),
        csel=np.ascontiguousarray(csel),
        att_src=np.ascontiguousarray(np.asarray(inputs["att_src"], np.float32)),
        att_dst=np.ascontiguousarray(np.asarray(inputs["att_dst"], np.float32)),
        gat_bias=np.asarray(inputs["gat_bias"], np.float32),
        w3cat4=w3c4,
        w5b=_conv_w5b(np.asarray(inputs["conv_w"], np.float32)),
        conv_b40=np.tile(np.asarray(inputs["conv_b"], np.float32), 5),
    )
    in_maps = []
    for k in range(NCORES):
        m = dict(common)
        nor = per_core[k]["node_of_rank"]
        xp = x[k * NL:(k + 1) * NL][nor]
        m["xperm"] = np.ascontiguousarray(
            xp.reshape(NBLK, 128, FIN).transpose(1, 0, 2).reshape(
                128, NBLK * FIN)).astype(np.float32)
        m["xe"] = per_core[k]["xe"]
        m["npad"] = per_core[k]["npad"]
        m["gidx"] = per_core[k]["gidx"]
        in_maps.append(m)

    import os
    trace = bool(os.environ.get("GAT_TRACE"))
    res = run_bass_kernel_spmd(nc, in_maps, list(range(NCORES)), trace=trace)
    global LAST_RESULT
    LAST_RESULT = res
    outs = []
    for k in range(NCORES):
        o = res.results[k]["out"]                     # [(r,o), (g,q)]
        o = o.reshape(5, COUT, GPC, QD).transpose(2, 1, 3, 0)  # g,o,q,r
        outs.append(o.reshape(GPC, COUT, TOUT))       # t = 5q + r
    return np.concatenate(outs, axis=0).astype(np.float32)


LAST_RESULT = None


if __name__ == "__main__":
    # smoke test with random data
    rng = np.random.default_rng(0)
    E = 3047424
    ins = dict(
        x=rng.standard_normal((N, FIN), dtype=np.float32),
        edge_index=rng.integers(0, N, size=(2, E), dtype=np.int64),
        batch=(np.arange(N, dtype=np.int64) // NPG),
        bn_gamma=np.ones(FIN, np.float32),
        bn_beta=np.zeros(FIN, np.float32),
        lin_w=rng.standard_normal((H * C, FIN), dtype=np.float32) * 0.447,
        att_src=rng.standard_normal((H, C), dtype=np.float32) * 0.1,
        att_dst=rng.standard_normal((H, C), dtype=np.float32) * 0.1,
        gat_bias=np.zeros(C, np.float32),
        conv_w=rng.standard_normal((COUT, C, KCONV), dtype=np.float32) * 0.05,
        conv_b=np.zeros(COUT, np.float32),
    )
    y = kernel(**ins)
    print(y.shape, y.dtype)


# revision 31
# speedup vs baseline: 1.0464x; 1.0464x over previous
"""GAT layer (BatchNorm -> GATConv -> head-mean -> ELU -> per-graph Conv1d)
on 8 Trainium2 NeuronCores via Bass/Tile.

v2 design: the edge gather is done host-side (pure index manipulation, like
the baseline's xperm/srcs prep): x[src] rows are laid out in the padded,
destination-rank-sorted edge stream order.  The device then:
  1. computes BN stats and folds BN + attention vectors into small
     broadcast constants (va, vd, ca+cd),
  2. streams the pre-gathered raw x edge rows with direct DMA and computes
     attention weights + accumulates per-destination segment sums as dense
     "round" adds (no indirect DMA in the main loop),
  3. corrects the padding slots' denominator contribution analytically
     (pad slots have x=0 so their exp(lrelu(adst2)) weight is computable
     densely per rank),
  4. normalizes, projects through a BN-folded weight matrix (block-diag x4
     for partition efficiency), adds bias, applies ELU, and
  5. writes normalized rows densely in rank order, gathers them into
     per-graph padded position order (190 indirect DMAs, issued upfront
     and pipelined with the projection), and runs the Conv1d as a
     residue-5 decomposition: t = 5q + r gives M=40 output rows x
     N=3200 columns in 14 K-passes of 120 (fp16 operands, fp32 psum).

All FP compute on tensor data happens on-device; host work is index
manipulation and pure weight-layout transforms.
"""

import sys

sys.path.insert(0, "/opt/trn_rl_repo")

import numpy as np
from contextlib import ExitStack

import concourse.bass as bass
import concourse.bacc as bacc
import concourse.tile as tile
from concourse import mybir
from concourse.masks import make_identity
from concourse.bass_utils import run_bass_kernel_spmd

F32 = mybir.dt.float32
BF16 = mybir.dt.float16
I32 = mybir.dt.int32
AF = mybir.ActivationFunctionType
OP = mybir.AluOpType

N = 190464
FIN = 5
H = 4
C = 24
NPG = 186          # nodes per graph
B = 1024           # graphs
NCORES = 8
GPC = B // NCORES  # 128 graphs per core
NL = N // NCORES   # 23808 local nodes per core
NBLK = NL // 128   # 186 rank blocks of 128
MB = N // 128      # 1488 nodes per partition in the flat x layout
KCONV = 62
COUT = 8
TOUT = NPG - KCONV + 1   # 125
PADG = 190               # per-graph padded length (5*38, conv shift head-room)
NPOS = GPC * PADG        # 24320 padded node positions per core
NPT = NPOS // 128        # 190 position tiles
G_CH = 186               # max edge tiles per main-loop chunk (whole rounds)
QD = TOUT // 5           # 25 output positions per residue
NP5 = 14                 # conv K passes (j = 5p + jj, j in 0..69)
EPS = 1e-5


# --------------------------------------------------------------------------
# host-side sharding / ordering (pure index manipulation)
# --------------------------------------------------------------------------
def _host_prep(edge_index, x):
    src_g = np.asarray(edge_index[0], dtype=np.int64)
    dst_g = np.asarray(edge_index[1], dtype=np.int64)
    xz = np.concatenate([np.asarray(x, np.float32),
                         np.zeros((1, FIN), np.float32)], axis=0)
    cores = []
    maxdeg = 0
    for k in range(NCORES):
        lo = k * NL
        m = (dst_g >= lo) & (dst_g < lo + NL)
        es = np.concatenate([src_g[m], np.arange(lo, lo + NL, dtype=np.int64)])
        ed = np.concatenate([dst_g[m] - lo, np.arange(NL, dtype=np.int64)])
        deg = np.bincount(ed, minlength=NL)
        node_of_rank = np.argsort(-deg, kind="stable")
        rank_of_node = np.empty(NL, dtype=np.int64)
        rank_of_node[node_of_rank] = np.arange(NL)
        r_e = rank_of_node[ed]
        perm = np.argsort(r_e, kind="stable")
        es_s = es[perm]
        r_s = r_e[perm]
        cnt = deg[node_of_rank]          # per-rank degree, descending
        starts = np.zeros(NL, dtype=np.int64)
        starts[1:] = np.cumsum(cnt)[:-1]
        j_s = np.arange(es_s.size, dtype=np.int64) - starts[r_s]
        maxdeg = max(maxdeg, int(cnt[0]))
        cores.append(dict(es_s=es_s, r_s=r_s, j_s=j_s, cnt=cnt,
                          node_of_rank=node_of_rank))

    # global (SPMD-identical) round sizes: K_j = #nodes with deg > j
    kmax = np.zeros(maxdeg, dtype=np.int64)
    for c in cores:
        kj = np.searchsorted(-c["cnt"], -np.arange(maxdeg), side="left")
        kmax = np.maximum(kmax, kj)
    r_tiles = (kmax + 127) // 128            # tiles per round
    r_edges = r_tiles * 128
    round_base = np.zeros(maxdeg + 1, dtype=np.int64)
    round_base[1:] = np.cumsum(r_edges)
    e_pad = int(round_base[-1])
    nt_total = e_pad // 128

    # chunk schedule (identical across cores): (t0, c0, nt)
    chunks = []
    t0 = 0
    for j in range(maxdeg):
        rem = int(r_tiles[j])
        c0 = 0
        while rem:
            nt = min(G_CH, rem)
            chunks.append((t0, c0, nt))
            t0 += nt
            c0 += nt
            rem -= nt
    assert t0 == nt_total

    per_core = []
    for c in cores:
        stream = np.full(e_pad, N, dtype=np.int64)    # zero row for padding
        pos = round_base[c["j_s"]] + c["r_s"]
        stream[pos] = c["es_s"]
        xe = xz[stream]                               # [e_pad, 5]
        xe = np.ascontiguousarray(
            xe.reshape(nt_total, 128, FIN).transpose(1, 0, 2).reshape(
                128, nt_total * FIN))

        # per-rank padding-slot count: rank r is padded in rounds
        # j in [cnt[r], maxdeg) where r < r_edges[j]
        npad = np.zeros(NL, dtype=np.int64)
        cnts = c["cnt"]
        for j in range(maxdeg):
            lo_r = np.searchsorted(-cnts, -j, side="left")  # first rank w/ cnt<=j
            hi_r = int(r_edges[j])
            if hi_r > lo_r:
                npad[lo_r:hi_r] += 1
        npad2 = np.ascontiguousarray(
            npad.reshape(NBLK, 128).T).astype(np.float32)

        nor = c["node_of_rank"]
        rank_of_node = np.empty(NL, dtype=np.int64)
        rank_of_node[nor] = np.arange(NL)
        gid = np.full(NPOS, NL, dtype=np.int64)       # pad -> zero row NL
        posa = np.arange(NPOS)
        g = posa // PADG
        s = posa % PADG
        real = s < NPG
        gid[real] = rank_of_node[g[real] * NPG + s[real]]
        gidx = np.ascontiguousarray(
            gid.reshape(NPT, 128).T).astype(np.int32)

        per_core.append(dict(xe=xe, npad=npad2, gidx=gidx,
                             node_of_rank=nor))

    amask = np.zeros((H * C, H), dtype=np.float32)
    for h in range(H):
        amask[h * C:(h + 1) * C, h] = 1.0
    return per_core, chunks, nt_total, amask


def _w3_layout20(lin_w):
    """Rows (h,f) = lin_w[h*24+c', f] (pure layout)."""
    w3 = np.zeros((4 * FIN, C), dtype=np.float32)
    for h in range(H):
        w3[h * FIN:(h + 1) * FIN, :] = lin_w[h * C:(h + 1) * C, :].T
    return w3


def _conv_w5b(cw):
    """conv_w[o, ci, k] -> w5b[(jj,ci), (p,(r,o))]: nonzero iff k=5p+jj-r
    in [0, KCONV). Pure layout."""
    w = np.zeros((5 * C, NP5 * 40), dtype=np.float32)
    for p in range(NP5):
        for jj in range(5):
            for r in range(5):
                k = 5 * p + jj - r
                if 0 <= k < KCONV:
                    w[jj * C:(jj + 1) * C, p * 40 + r * COUT:(p * 40 + (r + 1) * COUT)] = \
                        cw[:, :, k].T
    return w


# --------------------------------------------------------------------------
# device program
# --------------------------------------------------------------------------
def _build(nt_total, chunks):
    nc = bacc.Bacc(None, target_bir_lowering=False)
    x_d = nc.declare_dram_parameter("x", [N, FIN], F32, isOutput=False)
    xperm_d = nc.declare_dram_parameter("xperm", [128, NBLK * FIN], F32, False)
    xe_d = nc.declare_dram_parameter("xe", [128, nt_total * FIN], F32, False)
    npad_d = nc.declare_dram_parameter("npad", [128, NBLK], F32, False)
    gidx_d = nc.declare_dram_parameter("gidx", [128, NPT], I32, False)
    amask_d = nc.declare_dram_parameter("amask", [H * C, H], F32, False)
    gam_d = nc.declare_dram_parameter("bn_gamma", [FIN], F32, False)
    bet_d = nc.declare_dram_parameter("bn_beta", [FIN], F32, False)
    lw_d = nc.declare_dram_parameter("lin_w", [H * C, FIN], F32, False)
    lwT_d = nc.declare_dram_parameter("lin_wT", [FIN, H * C], F32, False)
    csel_d = nc.declare_dram_parameter("csel", [H * C, C], F32, False)
    asc_d = nc.declare_dram_parameter("att_src", [H, C], F32, False)
    adc_d = nc.declare_dram_parameter("att_dst", [H, C], F32, False)
    gb_d = nc.declare_dram_parameter("gat_bias", [C], F32, False)
    w3c4_d = nc.declare_dram_parameter("w3cat4", [80, 96], F32, False)
    w5b_d = nc.declare_dram_parameter("w5b", [5 * C, NP5 * 40], F32, False)
    cb40_d = nc.declare_dram_parameter("conv_b40", [40], F32, False)
    # out in [(r,o), (g,q)] layout; host de-interleaves t = 5q + r
    out_d = nc.declare_dram_parameter("out", [40, GPC * QD], F32,
                                      isOutput=True)

    spillA = nc.dram_tensor("spillA", [NL + 128, 4 * FIN], BF16)
    wscr = nc.dram_tensor("wscr", [FIN, 2 * H], F32)

    with tile.TileContext(nc) as tc, ExitStack() as ctx:
        cpool = ctx.enter_context(tc.tile_pool(name="const", bufs=1))

        # ---------- persistent constants ----------
        ident = cpool.tile([128, 128], F32)
        make_identity(nc, ident[:])
        ones_col = cpool.tile([128, 1], F32)
        nc.vector.memset(ones_col[:], 1.0)
        ones_row = cpool.tile([1, 128], F32)
        nc.vector.memset(ones_row[:], 1.0)

        acc20 = cpool.tile([128, NBLK * 4 * FIN], F32)
        nc.vector.memset(acc20[:], 0.0)
        accW = cpool.tile([128, NBLK * H], F32)
        nc.vector.memset(accW[:], 0.0)
        adst2 = cpool.tile([128, NBLK * H], F32)
        adst2h = cpool.tile([128, NBLK * H], BF16)
        vah = cpool.tile([128, 4 * FIN], BF16)

        gidx_sb = cpool.tile([128, NPT], I32)
        nc.sync.dma_start(out=gidx_sb[:], in_=gidx_d[:, :])
        npad_sb = cpool.tile([128, NBLK], F32)
        nc.sync.dma_start(out=npad_sb[:], in_=npad_d[:, :])

        identb = cpool.tile([128, 128], BF16)
        nc.vector.tensor_copy(out=identb[:], in_=ident[:])
        w3p4 = cpool.tile([80, 96], F32)       # BN-folded block-diag proj
        w3p4b = cpool.tile([80, 96], BF16)
        wc5f = cpool.tile([5 * C, NP5 * 40], F32)
        nc.sync.dma_start(out=wc5f[:], in_=w5b_d[:, :])
        wc5 = cpool.tile([5 * C, NP5 * 40], BF16)
        nc.vector.tensor_copy(out=wc5[:], in_=wc5f[:])
        cb40 = cpool.tile([40, 1], F32)
        nc.sync.dma_start(out=cb40[:], in_=cb40_d[:, None])
        cb4 = cpool.tile([96, 1], F32)         # gat out bias x4 (h-mean folded)

        b54 = cpool.tile([128, 54], F32)  # [s(5) t(5) va(20) vd(20) cacd(4)]

        # ---------- setup phase: BN stats, folded vectors, adst2 ----------
        with tc.tile_pool(name="setup", bufs=1) as spool, \
             tc.tile_pool(name="spsum", bufs=1, space="PSUM") as sps:
            # zero the dummy rows NL..NL+127 of spillA (pad positions)
            zpad = spool.tile([128, 4 * FIN], BF16)
            nc.vector.memset(zpad[:], 0.0)
            nc.sync.dma_start(out=spillA[NL:NL + 128, :], in_=zpad[:])

            xsb = spool.tile([128, MB * FIN], F32)
            nc.sync.dma_start(
                out=xsb[:],
                in_=x_d[:, :].rearrange("(p m) f -> p (m f)", p=128))
            tmp = spool.tile([128, MB * FIN], F32)

            # per-partition partial sums of x and x^2  -> [128, 10]
            xpart = spool.tile([128, 10], F32)
            nc.scalar.activation(out=tmp[:], in_=xsb[:], func=AF.Square)
            nc.vector.tensor_reduce(
                out=xpart[:, 0:FIN],
                in_=xsb[:].rearrange("p (m f) -> p f m", f=FIN),
                axis=mybir.AxisListType.X, op=OP.add)
            nc.vector.tensor_reduce(
                out=xpart[:, FIN:2 * FIN],
                in_=tmp[:].rearrange("p (m f) -> p f m", f=FIN),
                axis=mybir.AxisListType.X, op=OP.add)
            sums_ps = sps.tile([1, 10], F32)
            nc.tensor.matmul(out=sums_ps[:], lhsT=ones_col[:], rhs=xpart[:],
                             start=True, stop=True)

            st1 = spool.tile([1, 32], F32)
            w2 = spool.tile([1, 54], F32)
            nc.vector.tensor_copy(out=st1[:, 0:10], in_=sums_ps[:])
            nc.vector.tensor_scalar_mul(out=st1[:, 0:5], in0=st1[:, 0:5],
                                        scalar1=1.0 / N)          # mu
            nc.vector.tensor_scalar_mul(out=st1[:, 5:10], in0=st1[:, 5:10],
                                        scalar1=1.0 / N)          # E[x^2]
            nc.vector.tensor_tensor(out=st1[:, 10:15], in0=st1[:, 0:5],
                                    in1=st1[:, 0:5], op=OP.mult)  # mu^2
            nc.vector.tensor_tensor(out=st1[:, 10:15], in0=st1[:, 5:10],
                                    in1=st1[:, 10:15], op=OP.subtract)  # var
            nc.vector.tensor_scalar_add(out=st1[:, 15:20],
                                        in0=st1[:, 10:15], scalar1=EPS)
            nc.scalar.activation(out=st1[:, 15:20], in_=st1[:, 15:20],
                                 func=AF.Sqrt)
            nc.vector.reciprocal(out=st1[:, 10:15], in_=st1[:, 15:20])  # rstd
            gsb = spool.tile([1, FIN], F32)
            bsb = spool.tile([1, FIN], F32)
            nc.sync.dma_start(out=gsb[:], in_=gam_d[None, :])
            nc.sync.dma_start(out=bsb[:], in_=bet_d[None, :])
            nc.vector.tensor_tensor(out=w2[:, 0:5], in0=gsb[:],
                                    in1=st1[:, 10:15], op=OP.mult)  # s
            nc.vector.tensor_tensor(out=st1[:, 20:25], in0=st1[:, 0:5],
                                    in1=w2[:, 0:5], op=OP.mult)     # mu*s
            nc.vector.tensor_tensor(out=w2[:, 5:10], in0=bsb[:],
                                    in1=st1[:, 20:25], op=OP.subtract)  # t

            # wa / wd via block-masked attention matmul (as baseline)
            attfs = spool.tile([H * C, 1], F32)
            attfd = spool.tile([H * C, 1], F32)
            nc.sync.dma_start(out=attfs[:],
                              in_=asc_d[:, :].rearrange("h c -> (h c)")[:, None])
            nc.sync.dma_start(out=attfd[:],
                              in_=adc_d[:, :].rearrange("h c -> (h c)")[:, None])
            amk = spool.tile([H * C, H], F32)
            nc.sync.dma_start(out=amk[:], in_=amask_d[:, :])
            a2 = spool.tile([H * C, 2 * H], F32)
            nc.vector.tensor_tensor(out=a2[:, 0:H],
                                    in0=attfs[:].to_broadcast([H * C, H]),
                                    in1=amk[:], op=OP.mult)
            nc.vector.tensor_tensor(out=a2[:, H:2 * H],
                                    in0=attfd[:].to_broadcast([H * C, H]),
                                    in1=amk[:], op=OP.mult)
            lwsb = spool.tile([H * C, FIN], F32)
            nc.sync.dma_start(out=lwsb[:], in_=lw_d[:, :])
            wps = sps.tile([FIN, 2 * H], F32)
            nc.tensor.matmul(out=wps[:], lhsT=lwsb[:], rhs=a2[:],
                             start=True, stop=True)
            wsb = spool.tile([FIN, 2 * H], F32)
            nc.vector.tensor_copy(out=wsb[:], in_=wps[:])
            nc.sync.dma_start(out=wscr[:, :], in_=wsb[:])
            tc.strict_bb_all_engine_barrier()   # wscr DRAM RAW
            # reload wa/wd in (h major, f minor) order: addr = f*8 + hh
            wflat = wscr[:, :].rearrange("f hh -> (f hh)")
            wad = spool.tile([1, 40], F32)
            nc.sync.dma_start(
                out=wad[:, 0:20],
                in_=wflat[None, :].rearrange("p (f hh) -> p hh f", hh=8)[:, 0:4, :])
            nc.sync.dma_start(
                out=wad[:, 20:40],
                in_=wflat[None, :].rearrange("p (f hh) -> p hh f", hh=8)[:, 4:8, :])
            # va = wa*s, vd = wd*s (per-head, f minor)
            s_bc4 = w2[:, 0:5][:, None, :].to_broadcast([1, 4, 5])
            nc.vector.tensor_tensor(
                out=w2[:, 10:30].rearrange("p (h f) -> p h f", f=5),
                in0=wad[:, 0:20].rearrange("p (h f) -> p h f", f=5),
                in1=s_bc4, op=OP.mult)
            nc.vector.tensor_tensor(
                out=w2[:, 30:50].rearrange("p (h f) -> p h f", f=5),
                in0=wad[:, 20:40].rearrange("p (h f) -> p h f", f=5),
                in1=s_bc4, op=OP.mult)
            # ca+cd: (wa+wd)*t summed over f
            t_bc4 = w2[:, 5:10][:, None, :].to_broadcast([1, 4, 5])
            wsum = spool.tile([1, 20], F32)
            nc.vector.tensor_tensor(out=wsum[:], in0=wad[:, 0:20],
                                    in1=wad[:, 20:40], op=OP.add)
            nc.vector.tensor_tensor(
                out=wsum[:].rearrange("p (h f) -> p h f", f=5),
                in0=wsum[:].rearrange("p (h f) -> p h f", f=5),
                in1=t_bc4, op=OP.mult)
            nc.vector.tensor_reduce(
                out=w2[:, 50:54],
                in_=wsum[:].rearrange("p (h f) -> p h f", f=5),
                axis=mybir.AxisListType.X, op=OP.add)

            # broadcast [1,54] -> [128,54] with a K=1 matmul
            b54_ps = sps.tile([128, 54], F32)
            nc.tensor.matmul(out=b54_ps[:], lhsT=ones_row[:], rhs=w2[:],
                             start=True, stop=True)
            nc.vector.tensor_copy(out=b54[:], in_=b54_ps[:])

            # s80 partition vector (s pattern x16) via K=1 matmul
            s80row = spool.tile([1, 80], F32)
            nc.vector.tensor_copy(
                out=s80row[:].rearrange("p (i f) -> p i f", f=5),
                in_=w2[:, 0:5][:, None, :].to_broadcast([1, 16, 5]))
            s80ps = sps.tile([80, 1], F32)
            ones1 = spool.tile([1, 1], F32)
            nc.vector.memset(ones1[:], 1.0)
            nc.tensor.matmul(out=s80ps[:], lhsT=s80row[:], rhs=ones1[:],
                             start=True, stop=True)
            s80 = spool.tile([80, 1], F32)
            nc.vector.tensor_copy(out=s80[:], in_=s80ps[:])
            # w3p4 = w3cat4 * s80 (BN scale folded into projection)
            w3c4 = spool.tile([80, 96], F32)
            nc.sync.dma_start(out=w3c4[:], in_=w3c4_d[:, :])
            nc.vector.tensor_tensor(out=w3p4[:], in0=w3c4[:],
                                    in1=s80[:].to_broadcast([80, 96]),
                                    op=OP.mult)
            nc.vector.tensor_copy(out=w3p4b[:], in_=w3p4[:])

            # cbase = 0.25 * sum_h (lin_w[(h,c)] . t) + gat_bias
            tcolps = sps.tile([FIN, 1], F32)
            nc.tensor.matmul(out=tcolps[:], lhsT=w2[:, 5:10], rhs=ones1[:],
                             start=True, stop=True)
            tcol = spool.tile([FIN, 1], F32)
            nc.vector.tensor_copy(out=tcol[:], in_=tcolps[:])
            lwT = spool.tile([FIN, H * C], F32)
            nc.sync.dma_start(out=lwT[:], in_=lwT_d[:, :])
            lwtps = sps.tile([H * C, 1], F32)
            nc.tensor.matmul(out=lwtps[:], lhsT=lwT[:], rhs=tcol[:],
                             start=True, stop=True)
            lwt96 = spool.tile([H * C, 1], F32)
            nc.vector.tensor_copy(out=lwt96[:], in_=lwtps[:])
            cselsb = spool.tile([H * C, C], F32)
            nc.sync.dma_start(out=cselsb[:], in_=csel_d[:, :])
            cbps = sps.tile([C, 1], F32)
            nc.tensor.matmul(out=cbps[:], lhsT=cselsb[:], rhs=lwt96[:],
                             start=True, stop=True)
            gb24 = spool.tile([C, 1], F32)
            nc.sync.dma_start(out=gb24[:], in_=gb_d[:, None])
            cbase = spool.tile([C, 1], F32)
            nc.scalar.activation(out=cbase[:], in_=cbps[:], func=AF.Identity,
                                 scale=0.25, bias=gb24[:])
            for i in range(4):
                nc.sync.dma_start(out=cb4[i * C:(i + 1) * C, :], in_=cbase[:])

            # adst2[p, (blk,h)] = sum_f vd[h,f]*xperm[p,blk,f] + (ca+cd)[h]
            xpb = spool.tile([128, NBLK * FIN], F32)
            nc.sync.dma_start(out=xpb[:], in_=xperm_d[:, :])
            xp_v = xpb[:].rearrange("p (m f) -> p m f", f=FIN)
            for h in range(H):
                vd_bc = b54[:, 30 + FIN * h:30 + FIN * (h + 1)]
                vd_bc = vd_bc[:, None, :].to_broadcast([128, NBLK, FIN])
                nc.vector.tensor_tensor(
                    out=tmp[:, 0:NBLK * FIN].rearrange("p (m f) -> p m f", f=FIN),
                    in0=xp_v, in1=vd_bc, op=OP.mult)
                nc.vector.tensor_reduce(
                    out=adst2[:].rearrange("p (m h) -> p m h", h=H)[:, :, h],
                    in_=tmp[:, 0:NBLK * FIN].rearrange("p (m f) -> p m f", f=FIN),
                    axis=mybir.AxisListType.X, op=OP.add)
            cacd_bc = b54[:, 50:54][:, None, :].to_broadcast([128, NBLK, H])
            nc.vector.tensor_tensor(
                out=adst2[:].rearrange("p (m h) -> p m h", h=H),
                in0=adst2[:].rearrange("p (m h) -> p m h", h=H),
                in1=cacd_bc, op=OP.add)
            nc.vector.tensor_copy(out=adst2h[:], in_=adst2[:])
            nc.vector.tensor_copy(out=vah[:], in_=b54[:, 10:30])

        # ---------- main edge loop (no indirect DMA) ----------
        with tc.tile_pool(name="gat", bufs=3) as gpool, \
             tc.tile_pool(name="mwork", bufs=2) as mpool, \
             nc.allow_low_precision(reason="fp16 attention path, 2e-2 gate"):
            for (t0, c0, nt) in chunks:
                xef = gpool.tile([128, G_CH * FIN], F32, tag="xef")
                nc.sync.dma_start(out=xef[:, 0:nt * FIN],
                                  in_=xe_d[:, t0 * FIN:(t0 + nt) * FIN])
                xet = gpool.tile([128, G_CH * FIN], BF16, tag="xet")
                nc.vector.tensor_copy(out=xet[:, 0:nt * FIN],
                                      in_=xef[:, 0:nt * FIN])
                xev = xet[:].rearrange("p (g f) -> p g f", f=FIN)
                tm = mpool.tile([128, G_CH * 4 * FIN], BF16, tag="tm")
                tmv = tm[:].rearrange("p (g h f) -> p g h f", h=H, f=FIN)
                va_bc = vah[:].rearrange(
                    "p (h f) -> p h f", f=FIN)[:, None, :, :]
                nc.vector.tensor_tensor(
                    out=tmv[:, 0:nt],
                    in0=xev[:, 0:nt, None, :].to_broadcast([128, nt, H, FIN]),
                    in1=va_bc.to_broadcast([128, nt, H, FIN]),
                    op=OP.mult)
                zt = mpool.tile([128, G_CH * H], BF16, tag="zt")
                nc.vector.tensor_reduce(
                    out=zt[:, 0:nt * H],
                    in_=tm[:, 0:nt * 4 * FIN].rearrange(
                        "p (gh f) -> p gh f", f=FIN),
                    axis=mybir.AxisListType.X, op=OP.add)
                nc.vector.tensor_tensor(
                    out=zt[:, 0:nt * H], in0=zt[:, 0:nt * H],
                    in1=adst2h[:, c0 * H:(c0 + nt) * H], op=OP.add)
                wt = mpool.tile([128, G_CH * H], BF16, tag="wt")
                # leaky_relu(z, 0.2) = max(0.2*z, z)
                nc.vector.scalar_tensor_tensor(
                    out=wt[:, 0:nt * H], in0=zt[:, 0:nt * H], scalar=0.2,
                    in1=zt[:, 0:nt * H], op0=OP.mult, op1=OP.max)
                nc.scalar.activation(out=wt[:, 0:nt * H], in_=wt[:, 0:nt * H],
                                     func=AF.Exp)
                ut = mpool.tile([128, G_CH * 4 * FIN], BF16, tag="ut")
                uv = ut[:].rearrange("p (g h f) -> p g h f", h=H, f=FIN)
                wv = wt[:].rearrange("p (g h) -> p g h", h=H)
                nc.vector.tensor_tensor(
                    out=uv[:, 0:nt],
                    in0=wv[:, 0:nt, :, None].to_broadcast([128, nt, H, FIN]),
                    in1=xev[:, 0:nt, None, :].to_broadcast([128, nt, H, FIN]),
                    op=OP.mult)
                nc.vector.tensor_tensor(
                    out=acc20[:, c0 * 20:(c0 + nt) * 20],
                    in0=acc20[:, c0 * 20:(c0 + nt) * 20],
                    in1=ut[:, 0:nt * 20],
                    op=OP.add)
                nc.vector.tensor_tensor(
                    out=accW[:, c0 * H:(c0 + nt) * H],
                    in0=accW[:, c0 * H:(c0 + nt) * H],
                    in1=wt[:, 0:nt * H],
                    op=OP.add)

        # ---------- normalize + scatter to position order ----------
        with tc.tile_pool(name="m2", bufs=2) as m2pool:
            # subtract padding-slot weights: wpad = exp(lrelu(adst2))
            wp = m2pool.tile([128, NBLK * H], F32, tag="wp")
            nc.vector.scalar_tensor_tensor(
                out=wp[:], in0=adst2[:], scalar=0.2,
                in1=adst2[:], op0=OP.mult, op1=OP.max)
            nc.scalar.activation(out=wp[:], in_=wp[:], func=AF.Exp)
            wpv = wp[:].rearrange("p (m h) -> p m h", h=H)
            nc.vector.tensor_tensor(
                out=wpv,
                in0=wpv,
                in1=npad_sb[:][:, :, None].to_broadcast([128, NBLK, H]),
                op=OP.mult)
            nc.vector.tensor_tensor(out=accW[:], in0=accW[:], in1=wp[:],
                                    op=OP.subtract)
            # U' = acc20 * (0.25 / denom)
            rden = m2pool.tile([128, NBLK * H], F32, tag="rden")
            nc.vector.reciprocal(out=rden[:], in_=accW[:])
            rd_v = rden[:].rearrange("p (c h) -> p c h", h=H)
            acc_v = acc20[:].rearrange("p (c h f) -> p c h f", h=H, f=FIN)
            accb16 = m2pool.tile([128, NBLK * 4 * FIN], BF16, tag="accb16")
            nc.vector.scalar_tensor_tensor(
                out=acc_v, in0=acc_v, scalar=0.25,
                in1=rd_v[:, :, :, None].to_broadcast([128, NBLK, H, FIN]),
                op0=OP.mult, op1=OP.mult)
            nc.vector.tensor_copy(out=accb16[:], in_=acc20[:])
            # dense write in rank order; position gathers read it back
            nc.sync.dma_start(
                out=spillA[0:NL, :].rearrange("(m p) c -> p m c", p=128),
                in_=accb16[:].rearrange("p (m c) -> p m c", c=4 * FIN))

        tc.strict_bb_all_engine_barrier()   # spillA DRAM RAW before gathers

        # ---------- reload, project, ELU into conv layout; conv ----------
        with tc.tile_pool(name="tail", bufs=1) as tpool, \
             tc.tile_pool(name="tl2", bufs=3) as tl2:
            v5 = tpool.tile([5 * C, NPOS], BF16)
            rl = tpool.tile([128, NPT * 20], BF16)
            for m in range(NPT):
                nc.gpsimd.indirect_dma_start(
                    out=rl[:, m * 20:(m + 1) * 20],
                    out_offset=None,
                    in_=spillA[:, :],
                    in_offset=bass.IndirectOffsetOnAxis(
                        ap=gidx_sb[:, m:m + 1], axis=0),
                )
            with tc.tile_pool(name="tlps", bufs=3, space="PSUM") as tlps:
                for m0 in range(0, NPT, 4):
                    gm = min(4, NPT - m0)
                    tps = tlps.tile([80, 128], BF16, tag="tps")
                    nc.tensor.transpose(
                        out=tps[0:gm * 20, :],
                        in_=rl[:, m0 * 20:(m0 + gm) * 20],
                        identity=identb[:])
                    trs = tl2.tile([80, 128], BF16, tag="trs")
                    nc.vector.tensor_copy(out=trs[0:gm * 20, :],
                                          in_=tps[0:gm * 20, :])
                    pm = tlps.tile([96, 128], F32, tag="pm")
                    nc.tensor.matmul(out=pm[0:gm * C, :],
                                     lhsT=w3p4b[0:gm * 20, 0:gm * C],
                                     rhs=trs[0:gm * 20, :],
                                     start=True, stop=True)
                    t1 = tl2.tile([96, 128], BF16, tag="t1")
                    nc.scalar.activation(out=t1[0:gm * C, :],
                                         in_=pm[0:gm * C, :],
                                         func=AF.Identity,
                                         bias=cb4[0:gm * C, :])
                    # ELU = max(x,0) + exp(min(x,0)) - 1
                    rp = tl2.tile([96, 128], BF16, tag="rp")
                    nc.vector.tensor_scalar_max(out=rp[0:gm * C, :],
                                                in0=t1[0:gm * C, :],
                                                scalar1=0.0)
                    nc.vector.tensor_scalar_min(out=t1[0:gm * C, :],
                                                in0=t1[0:gm * C, :],
                                                scalar1=0.0)
                    nc.scalar.activation(out=t1[0:gm * C, :],
                                         in_=t1[0:gm * C, :], func=AF.Exp)
                    nc.vector.scalar_tensor_tensor(
                        out=t1[0:gm * C, :], in0=t1[0:gm * C, :], scalar=-1.0,
                        in1=rp[0:gm * C, :], op0=OP.add, op1=OP.add)
                    for i in range(gm):
                        eng = nc.sync if i % 2 == 0 else nc.scalar
                        eng.dma_start(
                            out=v5[0:C, (m0 + i) * 128:(m0 + i + 1) * 128],
                            in_=t1[i * C:(i + 1) * C, :])

            # 4 shifted copies for the (ci, jj) contraction rows, chunked
            # across the three DMA-capable engine queues
            dmaengs = [nc.sync, nc.scalar, nc.gpsimd]
            CHK = 5
            csz = (NPOS + CHK - 1) // CHK
            ei = 0
            for kk in range(1, 5):
                for cc in range(CHK):
                    a = cc * csz
                    b = min(NPOS - kk, (cc + 1) * csz)
                    if b <= a:
                        continue
                    dmaengs[ei % 3].dma_start(
                        out=v5[kk * C:(kk + 1) * C, a:b],
                        in_=v5[0:C, a + kk:b + kk])
                    ei += 1
            tc.strict_bb_all_engine_barrier()

            # conv: out[(r,o), (g,q)] accumulated over NP5 K-passes
            gsz = [20, 20, 20, 20, 20, 20, 8]
            g0s = [0, 20, 40, 60, 80, 100, 120]
            v5v = v5[:].rearrange("k (g t) -> k g t", t=PADG)
            with tc.tile_pool(name="cps", bufs=1, space="PSUM") as cps:
                pcs = []
                for ci in range(7):
                    pc_t = cps.tile([40, 512], F32, tag=f"pc{ci}")
                    pcs.append(pc_t)
                for p in range(NP5):
                    for ci in range(7):
                        g0, gn = g0s[ci], gsz[ci]
                        nc.tensor.matmul(
                            out=pcs[ci][:, 0:gn * QD],
                            lhsT=wc5[:, p * 40:(p + 1) * 40],
                            rhs=v5v[:, g0:g0 + gn, 5 * p:5 * p + 121:5],
                            start=(p == 0), stop=(p == NP5 - 1))
                for ci in range(7):
                    g0, gn = g0s[ci], gsz[ci]
                    osb = tl2.tile([40, 512], F32, tag="osb")
                    nc.scalar.activation(out=osb[:, 0:gn * QD],
                                         in_=pcs[ci][:, 0:gn * QD],
                                         func=AF.Lrelu, bias=cb40[:],
                                         alpha=0.01)
                    nc.sync.dma_start(
                        out=out_d[:, g0 * QD:(g0 + gn) * QD],
                        in_=osb[:, 0:gn * QD])

    nc.compile()
    return nc


# --------------------------------------------------------------------------
# entry point
# --------------------------------------------------------------------------
def kernel(**inputs):
    x = np.ascontiguousarray(np.asarray(inputs["x"], dtype=np.float32))
    edge_index = np.asarray(inputs["edge_index"])
    per_core, chunks, nt_total, amask = _host_prep(edge_index, x)

    nc = _build(nt_total, chunks)

    lin_w = np.ascontiguousarray(np.asarray(inputs["lin_w"], np.float32))
    w3c20 = _w3_layout20(lin_w)
    w3c4 = np.zeros((80, 96), np.float32)
    for i in range(4):
        w3c4[i * 20:(i + 1) * 20, i * 24:(i + 1) * 24] = w3c20
    csel = np.tile(np.eye(C, dtype=np.float32), (H, 1))
    common = dict(
        x=x,
        amask=amask,
        bn_gamma=np.asarray(inputs["bn_gamma"], np.float32),
        bn_beta=np.asarray(inputs["bn_beta"], np.float32),
        lin_w=lin_w,
        lin_wT=np.ascontiguousarray(lin_w.T),
        csel=np.ascontiguousarray(csel),
        att_src=np.ascontiguousarray(np.asarray(inputs["att_src"], np.float32)),
        att_dst=np.ascontiguousarray(np.asarray(inputs["att_dst"], np.float32)),
        gat_bias=np.asarray(inputs["gat_bias"], np.float32),
        w3cat4=w3c4,
        w5b=_conv_w5b(np.asarray(inputs["conv_w"], np.float32)),
        conv_b40=np.tile(np.asarray(inputs["conv_b"], np.float32), 5),
    )
    in_maps = []
    for k in range(NCORES):
        m = dict(common)
        nor = per_core[k]["node_of_rank"]
        xp = x[k * NL:(k + 1) * NL][nor]
        m["xperm"] = np.ascontiguousarray(
            xp.reshape(NBLK, 128, FIN).transpose(1, 0, 2).reshape(
                128, NBLK * FIN)).astype(np.float32)
        m["xe"] = per_core[k]["xe"]
        m["npad"] = per_core[k]["npad"]
        m["gidx"] = per_core[k]["gidx"]
        in_maps.append(m)

    import os
    trace = bool(os.environ.get("GAT_TRACE"))
    res = run_bass_kernel_spmd(nc, in_maps, list(range(NCORES)), trace=trace)
    global LAST_RESULT
    LAST_RESULT = res
    outs = []
    for k in range(NCORES):
        o = res.results[k]["out"]                     # [(r,o), (g,q)]
        o = o.reshape(5, COUT, GPC, QD).transpose(2, 1, 3, 0)  # g,o,q,r
        outs.append(o.reshape(GPC, COUT, TOUT))       # t = 5q + r
    return np.concatenate(outs, axis=0).astype(np.float32)


LAST_RESULT = None


if __name__ == "__main__":
    # smoke test with random data
    rng = np.random.default_rng(0)
    E = 3047424
    ins = dict(
        x=rng.standard_normal((N, FIN), dtype=np.float32),
        edge_index=rng.integers(0, N, size=(2, E), dtype=np.int64),
        batch=(np.arange(N, dtype=np.int64) // NPG),
        bn_gamma=np.ones(FIN, np.float32),
        bn_beta=np.zeros(FIN, np.float32),
        lin_w=rng.standard_normal((H * C, FIN), dtype=np.float32) * 0.447,
        att_src=rng.standard_normal((H, C), dtype=np.float32) * 0.1,
        att_dst=rng.standard_normal((H, C), dtype=np.float32) * 0.1,
        gat_bias=np.zeros(C, np.float32),
        conv_w=rng.standard_normal((COUT, C, KCONV), dtype=np.float32) * 0.05,
        conv_b=np.zeros(COUT, np.float32),
    )
    y = kernel(**ins)
    print(y.shape, y.dtype)
